# revision 15
# baseline (speedup 1.0000x reference)
"""DenoiseGAT Trainium2 kernel: 8-core data-parallel over polygons (cycle graphs).

Per core: 256 polygons x 64 nodes = 16384 nodes. Activations as h^T
(features x nodes, bf16), 256-row tensors stored as [128, 2, n] tiles
(half index on the free dim). Attention: scores via block-diag a-matmul;
softmax in poly-partition block layout; alpha replicated to feature rows
via DRAM-staged broadcast DMA; neighbor combine via +-1 shifted tensor
ops (shifts stay inside 64-node polygons).
"""

import numpy as np
import ml_dtypes
from contextlib import ExitStack

import concourse.bass as bass
import concourse.tile as tile
import concourse.tile_utils as tile_utils
from concourse import bacc, mybir
from concourse.bass_utils import run_bass_kernel_spmd

tile_utils.max_sbuf_usage = 208 * 1024

F32 = mybir.dt.float32
BF16 = mybir.dt.bfloat16
ALU = mybir.AluOpType
ACTF = mybir.ActivationFunctionType

NCORES = 8
B, V = 2048, 64
HID, TDIM = 256, 128
BC = B // NCORES            # 256 polygons / core
N = BC * V                  # 16384 nodes / core
NT = 512                    # matmul node tile
SCH = 8192                  # softmax chunk = 128 polys
CCH = 1024                  # combine subchunk = 16 polys


def _ablk(asrc, atgt):
    NH, FO = asrc.shape
    out = np.zeros((NH * FO, 2 * NH), np.float32)
    for h in range(NH):
        out[h * FO:(h + 1) * FO, h] = asrc[h]
        out[h * FO:(h + 1) * FO, NH + h] = atgt[h]
    return out


def _bf(a):
    return np.ascontiguousarray(np.asarray(a, np.float32).astype(ml_dtypes.bfloat16))


def _f32(a):
    return np.ascontiguousarray(np.asarray(a, np.float32))


def _poly(ap, v=V):
    return ap.rearrange("p (g v) -> p g v", v=v)


def build(weights):
    nc = bacc.Bacc("TRN2", target_bir_lowering=False, debug=False,
                   enable_asserts=False, num_devices=NCORES)
    w = weights

    def inl(name, arr):
        return nc.inline_tensor(np.ascontiguousarray(arr), name=name).ap()

    half = TDIM // 2
    freqs = np.exp(-np.log(10000.0) * np.arange(half, dtype=np.float32) / (half - 1))
    fr2 = np.stack([np.concatenate([freqs, freqs]),
                    np.concatenate([np.zeros(half, np.float32),
                                    np.full(half, np.pi / 2, np.float32)])])
    ph = np.arange(V, dtype=np.float32) * (2 * np.pi / V)
    posT = np.tile(np.stack([np.sin(ph), np.cos(ph), np.sin(2 * ph), np.cos(2 * ph)]), (1, BC))

    def half3(a):
        """(256, X) host -> (128, 2, X) so tile[:, j, :] == rows 128j:128j+128."""
        a = np.asarray(a)
        return np.ascontiguousarray(a.reshape(2, 128, a.shape[1]).transpose(1, 0, 2))

    W0 = _f32(w["W0"]); sk0 = _f32(w["skip0"]); ab0 = _ablk(_f32(w["asrc0"]), _f32(w["atgt0"]))
    c_fr2 = inl("fr2", fr2.astype(np.float32))
    c_tW = inl("tW", _f32(w["tW"]))
    c_tb = inl("tb", _f32(w["tb"]).reshape(-1, 1))
    c_posT = inl("posT", _bf(posT))
    c_Wsum0t = inl("Wsum0t", W0[6:] + sk0[6:])          # (128, 256)
    c_W0ab = inl("W0ab", W0[6:] @ ab0)                  # (128, 8)
    c_W0f = inl("W0f", _bf(np.concatenate([W0[:6], sk0[:6]], 1)))   # (6, 512)
    c_ab0 = inl("ab0", half3(_bf(ab0)))                 # (128, 2, 8)
    c_b0 = inl("b0c", half3(_f32(w["b0"]).reshape(-1, 1)))
    c_eye8 = inl("eye8", np.eye(8, dtype=np.float32))
    LW, LAB, LB = {}, {}, {}
    for i in (1, 2):
        LW[i] = inl(f"W{i}f", half3(_bf(np.concatenate([_f32(w[f"W{i}"]), _f32(w[f"skip{i}"])], 1))))
        LAB[i] = inl(f"ab{i}f", half3(_bf(_ablk(_f32(w[f"asrc{i}"]), _f32(w[f"atgt{i}"])))))
        LB[i] = inl(f"b{i}c", half3(_f32(w[f"b{i}"]).reshape(-1, 1)))
    c_W3 = inl("W3f", half3(_bf(_f32(w["W3"]))))
    c_ab3 = inl("ab3f", half3(_bf(_ablk(_f32(w["asrc3"]), _f32(w["atgt3"])))))
    c_b3 = inl("b3c", half3(_f32(w["b3"]).reshape(-1, 1)))
    c_h1W = inl("h1Wf", half3(_bf(_f32(w["h1W"]))))
    c_h1b = inl("h1bc", half3(_f32(w["h1b"]).reshape(-1, 1)))
    c_h2W = inl("h2Wf", half3(_bf(_f32(w["h2W"]))))
    c_h2b = inl("h2bc", _f32(w["h2b"]).reshape(-1, 1))

    xT = nc.dram_tensor("xT", [2, N], BF16, kind="ExternalInput").ap()
    tp = nc.dram_tensor("tp", [2, BC], F32, kind="ExternalInput").ap()
    yT = nc.dram_tensor("yT", [2, N], F32, kind="ExternalOutput").ap()

    with tile.TileContext(nc) as tc, ExitStack() as ctx:
        P = ctx.enter_context(tc.tile_pool(name="pers", bufs=1))
        WP = ctx.enter_context(tc.tile_pool(name="wts", bufs=1))
        DR = ctx.enter_context(tc.tile_pool(name="dram", bufs=1, space="DRAM"))
        PS = ctx.enter_context(tc.tile_pool(name="ps", bufs=5, space="PSUM"))
        PSC = ctx.enter_context(tc.tile_pool(name="pssc", bufs=3, space="PSUM"))
        SM = ctx.enter_context(tc.tile_pool(name="sm", bufs=1))
        CB = ctx.enter_context(tc.tile_pool(name="cb", bufs=1))
        SK = ctx.enter_context(tc.tile_pool(name="sk", bufs=2))
        PJ = ctx.enter_context(tc.tile_pool(name="pj", bufs=1))

        h = P.tile([128, 2, N], BF16, tag="h")
        tembT = P.tile([TDIM, BC], F32, tag="tembT")
        G0T = P.tile([128, 2, BC], F32, tag="G0T")
        s0gT2 = P.tile([128, 2, 8], F32, tag="s0gT2")

        def load(c_ap, tag):
            t = WP.tile(list(c_ap.shape), c_ap.dtype, tag=tag)
            nc.sync.dma_start(t[:], c_ap)
            return t

        t_eye8 = load(c_eye8, "eye8")
        t_fr2 = load(c_fr2, "fr2")
        t_tp = load(tp, "tp")
        ps_te = PSC.tile([TDIM, BC], F32, tag="psA")
        nc.tensor.matmul(ps_te[:], t_fr2[:], t_tp[:], start=True, stop=True)
        te_m = SM.tile([TDIM, BC], F32, tag="Sblk")
        te_q = SM.tile([TDIM, BC], mybir.dt.int32, tag="den")
        nc.vector.tensor_scalar(te_q[:], ps_te[:], float(1.0 / (2 * np.pi)), None, op0=ALU.mult)
        te_qf = SM.tile([TDIM, BC], F32, tag="rd")
        nc.vector.tensor_copy(te_qf[:], te_q[:])
        nc.vector.scalar_tensor_tensor(te_m[:], te_qf[:], float(-2 * np.pi), ps_te[:],
                                       op0=ALU.mult, op1=ALU.add)
        te_s = SM.tile([TDIM, BC], F32, tag="E")
        nc.scalar.activation(te_s[:], te_m[:], ACTF.Sin)
        t_tW = load(c_tW, "tW")
        t_tb = load(c_tb, "tb")
        ps_tm = PSC.tile([TDIM, BC], F32, tag="psA")
        nc.tensor.matmul(ps_tm[:], t_tW[:], te_s[:], start=True, stop=True)
        nc.scalar.activation(tembT[:], ps_tm[:], ACTF.Silu, bias=t_tb[:])

        t_Ws0 = load(c_Wsum0t, "Ws0")
        for m in range(2):
            ps_g = PSC.tile([128, BC], F32, tag="psA")
            nc.tensor.matmul(ps_g[:], t_Ws0[:, m * 128:(m + 1) * 128], tembT[:],
                             start=True, stop=True)
            nc.vector.tensor_copy(G0T[:, m, :], ps_g[:])
        t_W0ab = load(c_W0ab, "W0ab")
        ps_sg = PSC.tile([8, BC], F32, tag="psA")
        nc.tensor.matmul(ps_sg[:], t_W0ab[:], tembT[:], start=True, stop=True)
        s0g = SM.tile([8, BC], F32, tag="EX")
        nc.vector.tensor_copy(s0g[:], ps_sg[:])
        for m in range(2):
            ps_t = PSC.tile([128, 8], F32, tag="psA")
            nc.tensor.matmul(ps_t[:], s0g[:, m * 128:(m + 1) * 128], t_eye8[:],
                             is_transpose=True, start=True, stop=True)
            nc.vector.tensor_copy(s0gT2[:, m, :], ps_t[:])

        h0loc = PJ.tile([6, SCH], BF16, tag="h0loc")
        TT = nc.vector.tensor_tensor
        GT = nc.gpsimd.tensor_tensor
        STT = nc.vector.scalar_tensor_tensor

        def layer(li, FIN, R, FO, c_w, c_ab, c_bias, first_layer):
            FOW = R * FO                 # 256
            nmt = (2 * FOW if li < 3 else FOW) // 128   # output 128-blocks
            kt = (FIN + 127) // 128
            t_w = load(c_w, f"w{li}")    # (6,512) L0 else (128, 2, 512|256)
            t_ab = load(c_ab, f"ab{li}")  # (128, 2, 2R)
            t_b = load(c_bias, f"b{li}")  # (128, 2, 1)
            a_dram = DR.tile([3 * R, N], BF16, tag="a_dram")
            sc_dram = DR.tile([2 * R, N], BF16, tag="sc_dram")

            def lhs_w(k, m):
                kk = min(128, FIN - k * 128)
                if first_layer:
                    return t_w[0:kk, m * 128:(m + 1) * 128]
                return t_w[0:kk, k, m * 128:(m + 1) * 128]

            for ch in range(N // SCH):
                u0 = ch * SCH
                if first_layer:
                    nc.sync.dma_start(h0loc[0:2, :], xT[:, u0:u0 + SCH])
                    nc.sync.dma_start(h0loc[2:6, :], c_posT[:, u0:u0 + SCH])
                projc = PJ.tile([128, 2, SCH], BF16, tag="projc")
                skc = None
                if li < 3:
                    skc = PJ.tile([128, 2, SCH], BF16, tag="skc")
                scT = SM.tile([128, SCH // 4], BF16, tag="scT")

                for it in range(SCH // NT):
                    u = it * NT
                    pst = [PS.tile([128, NT], F32, tag="mm", name=f"mm{_m}") for _m in range(nmt)]
                    for m in range(nmt):
                        for k in range(kt):
                            kk = min(128, FIN - k * 128)
                            rhs = (h0loc[0:kk, u:u + NT] if first_layer
                                   else h[0:kk, k, u0 + u:u0 + u + NT])
                            nc.tensor.matmul(pst[m][:], lhs_w(k, m), rhs,
                                             start=(k == 0), stop=(k == kt - 1))
                    for m in range(nmt):
                        if m < FOW // 128:
                            nc.scalar.activation(projc[:, m, u:u + NT], pst[m][:], ACTF.Copy)
                        else:
                            nc.scalar.activation(skc[:, m - 2, u:u + NT], pst[m][:],
                                                 ACTF.Identity, bias=t_b[:, m - 2, :])
                    s = it % 4
                    if s == 0:
                        scp = PSC.tile([128, NT], F32, tag="psA")
                    for k in range(2):
                        nc.tensor.matmul(scp[32 * s:32 * s + 2 * R, :], t_ab[:, k, :],
                                         projc[:, k, u:u + NT], start=(k == 0), stop=(k == 1),
                                         tile_position=(0, 32 * s))
                    if s == 3:
                        g = it // 4
                        nc.scalar.activation(scT[:, g * NT:(g + 1) * NT], scp[:], ACTF.Copy)

                scd = sc_dram[:, u0:u0 + SCH].rearrange("r (cb s w) -> r cb s w", s=4, w=NT)
                for s in range(4):
                    src = scT[32 * s:32 * s + 2 * R, :].rearrange("p (cb w) -> p cb w", w=NT)
                    nc.sync.dma_start(scd[:, :, s, :], src)
                S = SM.tile([128, 2 * R * V], BF16, tag="Sblk")
                src = sc_dram[:, u0:u0 + SCH].rearrange("r (p v) -> p r v", v=V)
                nc.sync.dma_start(S[:].rearrange("p (r v) -> p r v", v=V), src)

                if first_layer:
                    gb = s0gT2[:, ch, :].unsqueeze(2).to_broadcast((128, 2 * R, V))
                    Sv = S[:].rearrange("p (r v) -> p r v", v=V)
                    TT(Sv, Sv, gb, op=ALU.add)

                E = SM.tile([128, 3 * R * V], BF16, tag="E")
                Sv = S[:].rearrange("p (r v) -> p r v", v=V)
                Ssrc, Stgt = Sv[:, 0:R, :], Sv[:, R:2 * R, :]
                Ev = E[:].rearrange("p (k r v) -> p k r v", k=3, v=V)
                TT(Ev[:, 0, :, 1:], Ssrc[:, :, :V - 1], Stgt[:, :, 1:], op=ALU.add)
                TT(Ev[:, 0, :, 0:1], Ssrc[:, :, V - 1:], Stgt[:, :, 0:1], op=ALU.add)
                TT(Ev[:, 1, :, :], Ssrc, Stgt, op=ALU.add)
                TT(Ev[:, 2, :, :V - 1], Ssrc[:, :, 1:], Stgt[:, :, :V - 1], op=ALU.add)
                TT(Ev[:, 2, :, V - 1:], Ssrc[:, :, 0:1], Stgt[:, :, V - 1:], op=ALU.add)
                STT(E[:], E[:], 0.2, E[:], op0=ALU.mult, op1=ALU.max)
                EX = SM.tile([128, 3 * R * V], BF16, tag="EX")
                nc.scalar.activation(EX[:], E[:], ACTF.Exp)
                den = SM.tile([128, R * V], F32, tag="den")
                TT(den[:], EX[:, 0:R * V], EX[:, R * V:2 * R * V], op=ALU.add)
                TT(den[:], den[:], EX[:, 2 * R * V:], op=ALU.add)
                rd = SM.tile([128, R * V], F32, tag="rd")
                nc.vector.reciprocal(rd[:], den[:])
                ab_blk = SM.tile([128, 3 * R * V], BF16, tag="ab_blk")
                for k in range(3):
                    TT(ab_blk[:, k * R * V:(k + 1) * R * V],
                       EX[:, k * R * V:(k + 1) * R * V], rd[:], op=ALU.mult)
                nc.sync.dma_start(
                    a_dram[:, u0:u0 + SCH].rearrange("j (p v) -> p j v", v=V),
                    ab_blk[:].rearrange("p (j v) -> p j v", v=V))

                blk = min(FO, 128)
                for sc in range(SCH // CCH):
                    v0 = sc * CCH
                    span = slice(u0 + v0, u0 + v0 + CCH)
                    af = [CB.tile([128, 2, CCH], BF16, tag=f"af{k}", name=f"af{k}") for k in range(3)]
                    for k in range(3):
                        for b0 in range(0, FOW, blk):
                            hh = b0 // FO
                            src = a_dram[k * R + hh:k * R + hh + 1, span]
                            nc.sync.dma_start(
                                af[k][b0 % 128:b0 % 128 + blk, b0 // 128, :],
                                src.to_broadcast((blk, CCH)))
                    C1 = CB.tile([128, 2, CCH], BF16, tag="C1")
                    C2 = CB.tile([128, 2, CCH], BF16, tag="C2")
                    C4 = CB.tile([128, 2, CCH], BF16, tag="C4")
                    for ht in range(2):
                        pjv = _poly(projc[:, ht, v0:v0 + CCH])
                        a0 = _poly(af[1][:, ht, :]); ap1 = _poly(af[2][:, ht, :])
                        am1 = _poly(af[0][:, ht, :])
                        c1 = _poly(C1[:, ht, :]); c2 = _poly(C2[:, ht, :]); c4 = _poly(C4[:, ht, :])
                        TT(c1, a0, pjv, op=ALU.mult)
                        TT(c2[:, :, :V - 1], ap1[:, :, :V - 1], pjv[:, :, 1:], op=ALU.mult)
                        TT(c2[:, :, V - 1:], ap1[:, :, V - 1:], pjv[:, :, 0:1], op=ALU.mult)
                        TT(c4[:, :, 1:], am1[:, :, 1:], pjv[:, :, :V - 1], op=ALU.mult)
                        TT(c4[:, :, 0:1], am1[:, :, 0:1], pjv[:, :, V - 1:], op=ALU.mult)
                    C3 = CB.tile([128, 2, CCH], BF16, tag="C3")
                    GT(C3[:], C1[:], C2[:], op=ALU.add)
                    pre = CB.tile([128, 2, CCH], BF16, tag="pre")
                    GT(pre[:], C3[:], C4[:], op=ALU.add)
                    if li < 3:
                        GT(pre[:], pre[:], skc[:, :, v0:v0 + CCH], op=ALU.add)
                        if first_layer:
                            g0 = (u0 + v0) // V
                            for ht in range(2):
                                gbh = G0T[:, ht, g0:g0 + CCH // V].unsqueeze(2).to_broadcast(
                                    (128, CCH // V, V))
                                pvh = _poly(pre[:, ht, :])
                                TT(pvh, pvh, gbh, op=ALU.add)
                        mn = CB.tile([128, 2, CCH], BF16, tag="C1")
                        nc.vector.tensor_scalar(mn[:], pre[:], 0.0, None, op0=ALU.min)
                        ex = CB.tile([128, 2, CCH], BF16, tag="C2")
                        nc.scalar.activation(ex[:], mn[:], ACTF.Exp)
                        rl = CB.tile([128, 2, CCH], BF16, tag="C4")
                        nc.vector.tensor_scalar(rl[:], pre[:], 0.0, None, op0=ALU.max)
                        STT(h[:, :, span], ex[:], -1.0, rl[:], op0=ALU.add, op1=ALU.add)
                    else:
                        out3 = CB.tile([128, 2, CCH], BF16, tag="C1")
                        for ht in range(2):
                            STT(out3[:, ht, :], pre[:, ht, :], t_b[:, ht, :],
                                h[:, ht, span], op0=ALU.add, op1=ALU.add)
                        nc.vector.tensor_copy(h[:, :, span], out3[:])

        layer(0, 6, 4, 64, c_W0f, c_ab0, c_b0, True)
        layer(1, 256, 4, 64, LW[1], LAB[1], LB[1], False)
        layer(2, 256, 4, 64, LW[2], LAB[2], LB[2], False)
        layer(3, 256, 1, 256, c_W3, c_ab3, c_b3, False)

        t_h1W = load(c_h1W, "h1W")
        t_h1b = load(c_h1b, "h1b")
        t_h2W = load(c_h2W, "h2W")
        t_h2b = load(c_h2b, "h2b")
        for it in range(N // NT):
            u = it * NT
            pst = [PS.tile([128, NT], F32, tag="mm", name=f"mmh{_m}") for _m in range(2)]
            for m in range(2):
                for k in range(2):
                    nc.tensor.matmul(pst[m][:], t_h1W[:, k, m * 128:(m + 1) * 128],
                                     h[:, k, u:u + NT], start=(k == 0), stop=(k == 1))
            h5 = CB.tile([128, 2, NT], BF16, tag="h5")
            for m in range(2):
                nc.scalar.activation(h5[:, m, :], pst[m][:], ACTF.Silu, bias=t_h1b[:, m, :])
            ps2 = PSC.tile([2, NT], F32, tag="psA")
            for k in range(2):
                nc.tensor.matmul(ps2[:], t_h2W[:, k, :], h5[:, k, :],
                                 start=(k == 0), stop=(k == 1))
            yst = SK.tile([2, NT], F32, tag="yst")
            nc.vector.tensor_scalar(yst[:], ps2[:], t_h2b[:], None, op0=ALU.add)
            nc.sync.dma_start(yT[:, u:u + NT], yst[:])

    nc.compile()
    return nc


def kernel(**inputs):
    x = np.asarray(inputs["x"], np.float32)
    t = np.asarray(inputs["t"])
    nc = build(inputs)
    in_maps = []
    for c in range(NCORES):
        xs = x[c * BC:(c + 1) * BC]
        xTs = np.ascontiguousarray(xs.reshape(N, 2).T).astype(ml_dtypes.bfloat16)
        ts = t[c * BC:(c + 1) * BC].astype(np.float32)
        tps = np.ascontiguousarray(np.stack([ts, np.ones_like(ts)]))
        in_maps.append({"xT": xTs, "tp": tps})
    res = run_bass_kernel_spmd(nc, in_maps, core_ids=list(range(NCORES)))
    outs = []
    for c in range(NCORES):
        yTs = res.results[c]["yT"]
        outs.append(yTs.T.reshape(BC, 2 * V).astype(np.float32))
    return np.concatenate(outs, 0)


# revision 18
# speedup vs baseline: 882.4396x; 882.4396x over previous
"""DenoiseGAT Trainium2 kernel: 8-core data-parallel over polygons (cycle graphs).

Per core: 256 polygons x 64 nodes = 16384 nodes. Activations as h^T
(features x nodes, bf16), 256-row tensors stored as [128, 2, n] tiles
(half index on the free dim). Attention: scores via block-diag a-matmul;
softmax in poly-partition block layout; alpha replicated to feature rows
via DRAM-staged broadcast DMA; neighbor combine via +-1 shifted tensor
ops (shifts stay inside 64-node polygons).
"""

import numpy as np
import ml_dtypes
from contextlib import ExitStack

import concourse.bass as bass
import concourse.tile as tile
import concourse.tile_utils as tile_utils
from concourse import bacc, mybir
from concourse.bass_utils import run_bass_kernel_spmd

tile_utils.max_sbuf_usage = 208 * 1024

F32 = mybir.dt.float32
BF16 = mybir.dt.bfloat16
ALU = mybir.AluOpType
ACTF = mybir.ActivationFunctionType

NCORES = 8
B, V = 2048, 64
HID, TDIM = 256, 128
BC = B // NCORES            # 256 polygons / core
N = BC * V                  # 16384 nodes / core
NT = 512                    # matmul node tile
SCH = 8192                  # softmax chunk = 128 polys
CCH = 1024                  # combine subchunk = 16 polys


def _ablk(asrc, atgt):
    NH, FO = asrc.shape
    out = np.zeros((NH * FO, 2 * NH), np.float32)
    for h in range(NH):
        out[h * FO:(h + 1) * FO, h] = asrc[h]
        out[h * FO:(h + 1) * FO, NH + h] = atgt[h]
    return out


def _bf(a):
    return np.ascontiguousarray(np.asarray(a, np.float32).astype(ml_dtypes.bfloat16))


def _f32(a):
    return np.ascontiguousarray(np.asarray(a, np.float32))


def _poly(ap, v=V):
    return ap.rearrange("p (g v) -> p g v", v=v)


def build(weights):
    nc = bacc.Bacc("TRN2", target_bir_lowering=False, debug=False,
                   enable_asserts=False, num_devices=NCORES)
    w = weights

    def inl(name, arr):
        return nc.inline_tensor(np.ascontiguousarray(arr), name=name).ap()

    half = TDIM // 2
    freqs = np.exp(-np.log(10000.0) * np.arange(half, dtype=np.float32) / (half - 1))
    fr2 = np.stack([np.concatenate([freqs, freqs]),
                    np.concatenate([np.zeros(half, np.float32),
                                    np.full(half, np.pi / 2, np.float32)])])
    ph = np.arange(V, dtype=np.float32) * (2 * np.pi / V)
    posT = np.tile(np.stack([np.sin(ph), np.cos(ph), np.sin(2 * ph), np.cos(2 * ph)]), (1, BC))

    def half3(a):
        """(256, X) host -> (128, 2, X) so tile[:, j, :] == rows 128j:128j+128."""
        a = np.asarray(a)
        return np.ascontiguousarray(a.reshape(2, 128, a.shape[1]).transpose(1, 0, 2))

    W0 = _f32(w["W0"]); sk0 = _f32(w["skip0"]); ab0 = _ablk(_f32(w["asrc0"]), _f32(w["atgt0"]))
    c_fr2 = inl("fr2", fr2.astype(np.float32))
    c_tW = inl("tW", _f32(w["tW"]))
    c_tb = inl("tb", _f32(w["tb"]).reshape(-1, 1))
    c_posT = inl("posT", _bf(posT))
    c_Wsum0t = inl("Wsum0t", W0[6:] + sk0[6:])          # (128, 256)
    c_W0ab = inl("W0ab", W0[6:] @ ab0)                  # (128, 8)
    c_W0f = inl("W0f", _bf(np.concatenate([W0[:6], sk0[:6]], 1)))   # (6, 512)
    c_ab0 = inl("ab0", half3(_bf(ab0)))                 # (128, 2, 8)
    c_b0 = inl("b0c", half3(_f32(w["b0"]).reshape(-1, 1)))
    c_eye8 = inl("eye8", np.eye(8, dtype=np.float32))
    LW, LAB, LB = {}, {}, {}
    for i in (1, 2):
        LW[i] = inl(f"W{i}f", half3(_bf(np.concatenate([_f32(w[f"W{i}"]), _f32(w[f"skip{i}"])], 1))))
        LAB[i] = inl(f"ab{i}f", half3(_bf(_ablk(_f32(w[f"asrc{i}"]), _f32(w[f"atgt{i}"])))))
        LB[i] = inl(f"b{i}c", half3(_f32(w[f"b{i}"]).reshape(-1, 1)))
    c_W3 = inl("W3f", half3(_bf(_f32(w["W3"]))))
    c_ab3 = inl("ab3f", half3(_bf(_ablk(_f32(w["asrc3"]), _f32(w["atgt3"])))))
    c_b3 = inl("b3c", half3(_f32(w["b3"]).reshape(-1, 1)))
    c_h1W = inl("h1Wf", half3(_bf(_f32(w["h1W"]))))
    c_h1b = inl("h1bc", half3(_f32(w["h1b"]).reshape(-1, 1)))
    c_h2W = inl("h2Wf", half3(_bf(_f32(w["h2W"]))))
    c_h2b = inl("h2bc", _f32(w["h2b"]).reshape(-1, 1))

    xT = nc.dram_tensor("xT", [2, N], BF16, kind="ExternalInput").ap()
    tp = nc.dram_tensor("tp", [2, BC], F32, kind="ExternalInput").ap()
    yT = nc.dram_tensor("yT", [2, N], F32, kind="ExternalOutput").ap()

    with tile.TileContext(nc) as tc, ExitStack() as ctx:
        P = ctx.enter_context(tc.tile_pool(name="pers", bufs=1))
        WP = ctx.enter_context(tc.tile_pool(name="wts", bufs=1))
        DR = ctx.enter_context(tc.tile_pool(name="dram", bufs=1, space="DRAM"))
        PS = ctx.enter_context(tc.tile_pool(name="ps", bufs=5, space="PSUM"))
        PSC = ctx.enter_context(tc.tile_pool(name="pssc", bufs=3, space="PSUM"))
        SM = ctx.enter_context(tc.tile_pool(name="sm", bufs=1))
        CB = ctx.enter_context(tc.tile_pool(name="cb", bufs=1))
        SK = ctx.enter_context(tc.tile_pool(name="sk", bufs=2))
        PJ = ctx.enter_context(tc.tile_pool(name="pj", bufs=1))

        h = P.tile([128, 2, N], BF16, tag="h")
        tembT = P.tile([TDIM, BC], F32, tag="tembT")
        G0T = P.tile([128, 2, BC], F32, tag="G0T")
        s0gT2 = P.tile([128, 2, 8], F32, tag="s0gT2")

        def load(c_ap, tag):
            t = WP.tile(list(c_ap.shape), c_ap.dtype, tag=tag)
            nc.sync.dma_start(t[:], c_ap)
            return t

        t_eye8 = load(c_eye8, "eye8")
        t_fr2 = load(c_fr2, "fr2")
        t_tp = load(tp, "tp")
        ps_te = PSC.tile([TDIM, BC], F32, tag="psA")
        nc.tensor.matmul(ps_te[:], t_fr2[:], t_tp[:], start=True, stop=True)
        te_m = SM.tile([TDIM, BC], F32, tag="Sblk")
        te_q = SM.tile([TDIM, BC], mybir.dt.int32, tag="den")
        nc.vector.tensor_scalar(te_q[:], ps_te[:], float(1.0 / (2 * np.pi)), None, op0=ALU.mult)
        te_qf = SM.tile([TDIM, BC], F32, tag="rd")
        nc.vector.tensor_copy(te_qf[:], te_q[:])
        nc.vector.scalar_tensor_tensor(te_m[:], te_qf[:], float(-2 * np.pi), ps_te[:],
                                       op0=ALU.mult, op1=ALU.add)
        te_s = SM.tile([TDIM, BC], F32, tag="E")
        nc.scalar.activation(te_s[:], te_m[:], ACTF.Sin)
        t_tW = load(c_tW, "tW")
        t_tb = load(c_tb, "tb")
        ps_tm = PSC.tile([TDIM, BC], F32, tag="psA")
        nc.tensor.matmul(ps_tm[:], t_tW[:], te_s[:], start=True, stop=True)
        nc.scalar.activation(tembT[:], ps_tm[:], ACTF.Silu, bias=t_tb[:])

        t_Ws0 = load(c_Wsum0t, "Ws0")
        for m in range(2):
            ps_g = PSC.tile([128, BC], F32, tag="psA")
            nc.tensor.matmul(ps_g[:], t_Ws0[:, m * 128:(m + 1) * 128], tembT[:],
                             start=True, stop=True)
            nc.vector.tensor_copy(G0T[:, m, :], ps_g[:])
        t_W0ab = load(c_W0ab, "W0ab")
        ps_sg = PSC.tile([8, BC], F32, tag="psA")
        nc.tensor.matmul(ps_sg[:], t_W0ab[:], tembT[:], start=True, stop=True)
        s0g = SM.tile([8, BC], F32, tag="EX")
        nc.vector.tensor_copy(s0g[:], ps_sg[:])
        for m in range(2):
            ps_t = PSC.tile([128, 8], F32, tag="psA")
            nc.tensor.matmul(ps_t[:], s0g[:, m * 128:(m + 1) * 128], t_eye8[:],
                             is_transpose=True, start=True, stop=True)
            nc.vector.tensor_copy(s0gT2[:, m, :], ps_t[:])

        h0loc = PJ.tile([6, SCH], BF16, tag="h0loc")
        TT = nc.vector.tensor_tensor
        GT = nc.gpsimd.tensor_tensor
        STT = nc.vector.scalar_tensor_tensor

        def layer(li, FIN, R, FO, c_w, c_ab, c_bias, first_layer):
            FOW = R * FO                 # 256
            nmt = (2 * FOW if li < 3 else FOW) // 128   # output 128-blocks
            kt = (FIN + 127) // 128
            t_w = load(c_w, f"w{li}")    # (6,512) L0 else (128, 2, 512|256)
            t_ab = load(c_ab, f"ab{li}")  # (128, 2, 2R)
            t_b = load(c_bias, f"b{li}")  # (128, 2, 1)
            a_dram = DR.tile([3 * R, N], BF16, tag="a_dram")
            sc_dram = DR.tile([2 * R, N], BF16, tag="sc_dram")

            def lhs_w(k, m):
                kk = min(128, FIN - k * 128)
                if first_layer:
                    return t_w[0:kk, m * 128:(m + 1) * 128]
                return t_w[0:kk, k, m * 128:(m + 1) * 128]

            for ch in range(N // SCH):
                u0 = ch * SCH
                if first_layer:
                    nc.sync.dma_start(h0loc[0:2, :], xT[:, u0:u0 + SCH])
                    nc.sync.dma_start(h0loc[2:6, :], c_posT[:, u0:u0 + SCH])
                projc = PJ.tile([128, 2, SCH], BF16, tag="projc")
                skc = None
                if li < 3:
                    skc = PJ.tile([128, 2, SCH], BF16, tag="skc")
                scT = SM.tile([128, SCH // 4], BF16, tag="scT")

                for it in range(SCH // NT):
                    u = it * NT
                    pst = [PS.tile([128, NT], F32, tag="mm", name=f"mm{_m}") for _m in range(nmt)]
                    for m in range(nmt):
                        for k in range(kt):
                            kk = min(128, FIN - k * 128)
                            rhs = (h0loc[0:kk, u:u + NT] if first_layer
                                   else h[0:kk, k, u0 + u:u0 + u + NT])
                            nc.tensor.matmul(pst[m][:], lhs_w(k, m), rhs,
                                             start=(k == 0), stop=(k == kt - 1))
                    for m in range(nmt):
                        if m < FOW // 128:
                            nc.scalar.activation(projc[:, m, u:u + NT], pst[m][:], ACTF.Copy)
                        else:
                            nc.scalar.activation(skc[:, m - 2, u:u + NT], pst[m][:],
                                                 ACTF.Identity, bias=t_b[:, m - 2, :])
                    s = it % 4
                    if s == 0:
                        scp = PSC.tile([128, NT], F32, tag="psA")
                    for k in range(2):
                        nc.tensor.matmul(scp[32 * s:32 * s + 2 * R, :], t_ab[:, k, :],
                                         projc[:, k, u:u + NT], start=(k == 0), stop=(k == 1),
                                         tile_position=(0, 32 * s))
                    if s == 3:
                        g = it // 4
                        nc.scalar.activation(scT[:, g * NT:(g + 1) * NT], scp[:], ACTF.Copy)

                scd = sc_dram[:, u0:u0 + SCH].rearrange("r (cb s w) -> r cb s w", s=4, w=NT)
                for s in range(4):
                    src = scT[32 * s:32 * s + 2 * R, :].rearrange("p (cb w) -> p cb w", w=NT)
                    nc.sync.dma_start(scd[:, :, s, :], src)
                S = SM.tile([128, 2 * R * V], BF16, tag="Sblk")
                src = sc_dram[:, u0:u0 + SCH].rearrange("r (p v) -> p r v", v=V)
                nc.sync.dma_start(S[:].rearrange("p (r v) -> p r v", v=V), src)

                if first_layer:
                    gb = s0gT2[:, ch, :].unsqueeze(2).to_broadcast((128, 2 * R, V))
                    Sv = S[:].rearrange("p (r v) -> p r v", v=V)
                    TT(Sv, Sv, gb, op=ALU.add)

                E = SM.tile([128, 3 * R * V], BF16, tag="E")
                Sv = S[:].rearrange("p (r v) -> p r v", v=V)
                Ssrc, Stgt = Sv[:, 0:R, :], Sv[:, R:2 * R, :]
                Ev = E[:].rearrange("p (k r v) -> p k r v", k=3, v=V)
                TT(Ev[:, 0, :, 1:], Ssrc[:, :, :V - 1], Stgt[:, :, 1:], op=ALU.add)
                TT(Ev[:, 0, :, 0:1], Ssrc[:, :, V - 1:], Stgt[:, :, 0:1], op=ALU.add)
                TT(Ev[:, 1, :, :], Ssrc, Stgt, op=ALU.add)
                TT(Ev[:, 2, :, :V - 1], Ssrc[:, :, 1:], Stgt[:, :, :V - 1], op=ALU.add)
                TT(Ev[:, 2, :, V - 1:], Ssrc[:, :, 0:1], Stgt[:, :, V - 1:], op=ALU.add)
                STT(E[:], E[:], 0.2, E[:], op0=ALU.mult, op1=ALU.max)
                EX = SM.tile([128, 3 * R * V], BF16, tag="EX")
                nc.scalar.activation(EX[:], E[:], ACTF.Exp)
                den = SM.tile([128, R * V], F32, tag="den")
                TT(den[:], EX[:, 0:R * V], EX[:, R * V:2 * R * V], op=ALU.add)
                TT(den[:], den[:], EX[:, 2 * R * V:], op=ALU.add)
                rd = SM.tile([128, R * V], F32, tag="rd")
                nc.vector.reciprocal(rd[:], den[:])
                ab_blk = SM.tile([128, 3 * R * V], BF16, tag="ab_blk")
                for k in range(3):
                    TT(ab_blk[:, k * R * V:(k + 1) * R * V],
                       EX[:, k * R * V:(k + 1) * R * V], rd[:], op=ALU.mult)
                nc.sync.dma_start(
                    a_dram[:, u0:u0 + SCH].rearrange("j (p v) -> p j v", v=V),
                    ab_blk[:].rearrange("p (j v) -> p j v", v=V))

                blk = min(FO, 128)
                for sc in range(SCH // CCH):
                    v0 = sc * CCH
                    span = slice(u0 + v0, u0 + v0 + CCH)
                    af = [CB.tile([128, 2, CCH], BF16, tag=f"af{k}", name=f"af{k}") for k in range(3)]
                    for k in range(3):
                        for b0 in range(0, FOW, blk):
                            hh = b0 // FO
                            src = a_dram[k * R + hh:k * R + hh + 1, span]
                            nc.sync.dma_start(
                                af[k][b0 % 128:b0 % 128 + blk, b0 // 128, :],
                                src.to_broadcast((blk, CCH)))
                    C1 = CB.tile([128, 2, CCH], BF16, tag="C1")
                    C2 = CB.tile([128, 2, CCH], BF16, tag="C2")
                    C4 = CB.tile([128, 2, CCH], BF16, tag="C4")
                    for ht in range(2):
                        pjv = _poly(projc[:, ht, v0:v0 + CCH])
                        a0 = _poly(af[1][:, ht, :]); ap1 = _poly(af[2][:, ht, :])
                        am1 = _poly(af[0][:, ht, :])
                        c1 = _poly(C1[:, ht, :]); c2 = _poly(C2[:, ht, :]); c4 = _poly(C4[:, ht, :])
                        TT(c1, a0, pjv, op=ALU.mult)
                        GT(c2[:, :, :V - 1], ap1[:, :, :V - 1], pjv[:, :, 1:], op=ALU.mult)
                        GT(c2[:, :, V - 1:], ap1[:, :, V - 1:], pjv[:, :, 0:1], op=ALU.mult)
                        TT(c4[:, :, 1:], am1[:, :, 1:], pjv[:, :, :V - 1], op=ALU.mult)
                        TT(c4[:, :, 0:1], am1[:, :, 0:1], pjv[:, :, V - 1:], op=ALU.mult)
                    C3 = CB.tile([128, 2, CCH], BF16, tag="C3")
                    TT(C3[:], C1[:], C4[:], op=ALU.add)
                    pre = CB.tile([128, 2, CCH], BF16, tag="pre")
                    GT(pre[:], C3[:], C2[:], op=ALU.add)
                    if li < 3:
                        GT(pre[:], pre[:], skc[:, :, v0:v0 + CCH], op=ALU.add)
                        if first_layer:
                            g0 = (u0 + v0) // V
                            for ht in range(2):
                                gbh = G0T[:, ht, g0:g0 + CCH // V].unsqueeze(2).to_broadcast(
                                    (128, CCH // V, V))
                                pvh = _poly(pre[:, ht, :])
                                TT(pvh, pvh, gbh, op=ALU.add)
                        mn = CB.tile([128, 2, CCH], BF16, tag="C1")
                        nc.vector.tensor_scalar(mn[:], pre[:], 0.0, None, op0=ALU.min)
                        ex = CB.tile([128, 2, CCH], BF16, tag="C2")
                        nc.scalar.activation(ex[:], mn[:], ACTF.Exp)
                        rl = CB.tile([128, 2, CCH], BF16, tag="C4")
                        nc.vector.tensor_scalar(rl[:], pre[:], 0.0, None, op0=ALU.max)
                        STT(h[:, :, span], ex[:], -1.0, rl[:], op0=ALU.add, op1=ALU.add)
                    else:
                        out3 = CB.tile([128, 2, CCH], BF16, tag="C1")
                        for ht in range(2):
                            STT(out3[:, ht, :], pre[:, ht, :], t_b[:, ht, :],
                                h[:, ht, span], op0=ALU.add, op1=ALU.add)
                        nc.vector.tensor_copy(h[:, :, span], out3[:])

        layer(0, 6, 4, 64, c_W0f, c_ab0, c_b0, True)
        layer(1, 256, 4, 64, LW[1], LAB[1], LB[1], False)
        layer(2, 256, 4, 64, LW[2], LAB[2], LB[2], False)
        layer(3, 256, 1, 256, c_W3, c_ab3, c_b3, False)

        t_h1W = load(c_h1W, "h1W")
        t_h1b = load(c_h1b, "h1b")
        t_h2W = load(c_h2W, "h2W")
        t_h2b = load(c_h2b, "h2b")
        for it in range(N // NT):
            u = it * NT
            pst = [PS.tile([128, NT], F32, tag="mm", name=f"mmh{_m}") for _m in range(2)]
            for m in range(2):
                for k in range(2):
                    nc.tensor.matmul(pst[m][:], t_h1W[:, k, m * 128:(m + 1) * 128],
                                     h[:, k, u:u + NT], start=(k == 0), stop=(k == 1))
            h5 = CB.tile([128, 2, NT], BF16, tag="h5")
            for m in range(2):
                nc.scalar.activation(h5[:, m, :], pst[m][:], ACTF.Silu, bias=t_h1b[:, m, :])
            ps2 = PSC.tile([2, NT], F32, tag="psA")
            for k in range(2):
                nc.tensor.matmul(ps2[:], t_h2W[:, k, :], h5[:, k, :],
                                 start=(k == 0), stop=(k == 1))
            yst = SK.tile([2, NT], F32, tag="yst")
            nc.vector.tensor_scalar(yst[:], ps2[:], t_h2b[:], None, op0=ALU.add)
            nc.sync.dma_start(yT[:, u:u + NT], yst[:])

    nc.compile()
    return nc


def kernel(**inputs):
    x = np.asarray(inputs["x"], np.float32)
    t = np.asarray(inputs["t"])
    nc = build(inputs)
    in_maps = []
    for c in range(NCORES):
        xs = x[c * BC:(c + 1) * BC]
        xTs = np.ascontiguousarray(xs.reshape(N, 2).T).astype(ml_dtypes.bfloat16)
        ts = t[c * BC:(c + 1) * BC].astype(np.float32)
        tps = np.ascontiguousarray(np.stack([ts, np.ones_like(ts)]))
        in_maps.append({"xT": xTs, "tp": tps})
    res = run_bass_kernel_spmd(nc, in_maps, core_ids=list(range(NCORES)))
    outs = []
    for c in range(NCORES):
        yTs = res.results[c]["yT"]
        outs.append(yTs.T.reshape(BC, 2 * V).astype(np.float32))
    return np.concatenate(outs, 0)
